# revision 11
# baseline (speedup 1.0000x reference)
"""GTE contrastive loss kernel for 8 Trainium2 NeuronCores.

Math (reference): loss = -mean_i( cos(a_i,p_i)/T - log(partition_i) ),
partition_i = rowsum_i(E_ap) + rowsum_i(E_aa) + colsum_i(E_ap)
            + colsum_i(E_pp) - 2*exp(1/T),   E_xy = exp(cos/T).

Device does only the O(N^2) work: bf16 sims on PE, exp on ACT (plus a
tunable slice on DVE via the Schraudolph int16 bit-trick), per-row sums
via the ACT accumulator, and bf16 esum compression (sum over the 8 row
tiles of each block) so the host can finish column sums.  Everything
O(N*D) — normalization, the pos_logit diagonal, colsum reduction over
partitions, log/mean — runs on the host.

Sharding: core k owns row block k (1024 rows); inputs are rotated by
-1024k rows so one SPMD program serves all cores.  Per core, 17 block
matrices of [1024, 1024]: 8 ap, aa 0-3, pp 0-3, and "block 4" of aa
(cores 0-3) or pp (cores 4-7), fed via a third input q so the program
stays identical across cores.  Symmetry routing: rowsums of aa/pp
blocks 1-3 are emitted per core, their colsums serve the other cores'
missing blocks 5-7; block 0 is the masked diagonal block (rowsum only);
block 4 is computed once globally per unordered pair, colsum serving
the opposite core's rowsum.
"""

import os
import sys

import numpy as np

for _p in ("/opt/trn_rl_repo", os.path.expanduser("/root/.axon_site/_ro/trn_rl_repo")):
    if os.path.isdir(_p) and _p not in sys.path:
        sys.path.insert(0, _p)

import ml_dtypes  # noqa: E402

from concourse import bass, masks, tile  # noqa: E402
from concourse.bass_utils import run_bass_kernel_spmd  # noqa: E402

mybir = bass.mybir
F32 = mybir.dt.float32
BF16 = mybir.dt.bfloat16
I16 = mybir.dt.int16

N, D, NCORES = 8192, 64, 8
B = N // NCORES            # 1024 rows per core
MT = B // 128              # 8 row tiles of 128
INV_T = 20.0

# Schraudolph exp on DVE: bf16 bits = round(logit * 128/ln2 + BIAS).
# BIAS offset -7.5 zeroes the mean relative error of exp-sums (calibrated
# against round-to-nearest int16 conversion semantics measured on device).
SCH_MUL = float(INV_T * 128.0 / np.log(2.0))
SCH_BIAS = 127.0 * 128.0 - 7.5

# Per-core job list: 8 paired column blocks + 1 single.  Each entry is a
# tuple of block names; "aaJ"/"ppJ" = block J of aa/pp, "apJ" = block J
# of ap, "q4" = block 4 of aa-or-pp via the q input.
PAIRS = [
    ("aa0", "aa1"),
    ("aa2", "aa3"),
    ("pp0", "pp1"),
    ("pp2", "pp3"),
    ("q4", "ap0"),
    ("ap1", "ap2"),
    ("ap3", "ap4"),
    ("ap5", "ap6"),
    ("ap7",),
]
NSTREAM = len(PAIRS)       # 9 rowsum streams

# Blocks whose colsums the host needs, in es-output order.
COLSUM_BLOCKS = ["aa1", "aa2", "aa3", "pp1", "pp2", "pp3", "q4",
                 "ap0", "ap1", "ap2", "ap3", "ap4", "ap5", "ap6", "ap7"]
ES_W = len(COLSUM_BLOCKS) * B

# (pair_idx, m) tiles whose exp runs on DVE (Schraudolph) instead of ACT.
# Tunable: offload enough to balance ACT vs DVE busy time.
SCH_TILES = {(7, m) for m in range(MT)} | {(6, m) for m in range(4, MT)}
# aa0/pp0 carry the masked diagonal; keep those pairs on ACT (the -100
# mask would overflow the int16 Schraudolph path).
assert all(p not in (0, 2) for (p, m) in SCH_TILES)

# Pairs whose rowsums come from the esum accumulator (scalar_tensor_tensor
# accum_out = running rowsum of the esum; host diffs consecutive m) instead
# of the ACT accumulator.  Requires the esum to span the full pair width,
# i.e. every block in the pair emits a colsum.
STT_PAIRS = {4, 5, 6, 7, 8}


def _blk(name):
    """(stat_kind, mov_kind, j): operand sources + column block index."""
    if name == "q4":
        return ("q", "q", 4)
    j = int(name[-1])
    if name.startswith("aa"):
        return ("a", "a", j)
    if name.startswith("pp"):
        return ("p", "p", j)
    return ("a", "p", j)  # apJ


def build_program():
    nc = bass.Bass()
    # Host-normalized bf16 inputs, rotated by -1024k rows per core.
    an_in = nc.declare_dram_parameter("an", [4 * B, D], BF16, isOutput=False)
    pn_in = nc.declare_dram_parameter("pn", [N, D], BF16, isOutput=False)
    qn_in = nc.declare_dram_parameter("qn", [2 * B, D], BF16, isOutput=False)
    o_st = nc.declare_dram_parameter("st", [128, MT * NSTREAM], F32,
                                     isOutput=True)
    o_es = nc.declare_dram_parameter("es", [128, ES_W], BF16, isOutput=True)

    es_off = {b: i * B for i, b in enumerate(COLSUM_BLOCKS)}

    with tile.TileContext(nc) as tc:
        import contextlib

        with contextlib.ExitStack() as ctx:
            res = ctx.enter_context(tc.tile_pool(name="results", bufs=1))
            st = res.tile([128, MT * NSTREAM], F32)

            ident0 = res.tile([128, 128], F32)
            masks.make_identity(nc, ident0[:])
            ident = res.tile([128, 128], BF16)
            nc.vector.tensor_copy(ident[:], ident0[:])
            # -100 on the diagonal: exp(20*(s-100)) == 0 in bf16, removing
            # the aa/pp self-terms on device (no e^20 cancellation on host,
            # which reduced matmul precision cannot support)
            msk = res.tile([128, 128], F32)
            nc.vector.tensor_scalar_mul(msk[:], ident0[:], -100.0)

            # normalized bf16 token tiles, [partition, t, d], token=t*128+p
            nat = ctx.enter_context(tc.tile_pool(name="nat", bufs=1))
            an_nat = nat.tile([128, 4 * MT, D], BF16)
            pn_nat = nat.tile([128, 8 * MT, D], BF16)
            qn_nat = nat.tile([128, 2 * MT, D], BF16)
            aT = nat.tile([64, 4 * B], BF16)
            pT = nat.tile([64, N], BF16)
            qT = nat.tile([64, 2 * B], BF16)
            an_src = an_in[:].rearrange("(t p) d -> p t d", p=128)
            pn_src = pn_in[:].rearrange("(t p) d -> p t d", p=128)
            qn_src = qn_in[:].rearrange("(t p) d -> p t d", p=128)

            # input DMAs, in order of first use
            nc.sync.dma_start(out=an_nat[:, 0:16, :], in_=an_src[:, 0:16, :])
            nc.sync.dma_start(out=an_nat[:, 16:32, :], in_=an_src[:, 16:32, :])
            nc.sync.dma_start(out=pn_nat[:, 0:16, :], in_=pn_src[:, 0:16, :])
            nc.sync.dma_start(out=qn_nat[:], in_=qn_src[:])
            nc.sync.dma_start(out=pn_nat[:, 16:64, :], in_=pn_src[:, 16:64, :])

            mmp = ctx.enter_context(tc.tile_pool(name="mm", bufs=2,
                                                 space="PSUM"))
            ep = ctx.enter_context(tc.tile_pool(name="etile", bufs=3))
            sump = ctx.enter_context(tc.tile_pool(name="esum", bufs=2))
            dummy = res.tile([128, 2 * B], BF16)

            def emit_T(dst, natT, tstart, tcount, dst_off):
                """PE-transpose tcount token tiles into dst[:, dst_off...]
                via a PSUM staging tile shared with the mm ring."""
                for q0 in range(0, tcount, 16):
                    n_t = min(16, tcount - q0)
                    tr = mmp.tile([64, 16 * 128], BF16, tag="mm", name="tr")
                    for s in range(n_t):
                        t = tstart + q0 + s
                        nc.tensor.transpose(tr[:, s * 128:(s + 1) * 128],
                                            natT[:, t, :], ident[:])
                    nc.vector.tensor_copy(
                        dst[:, dst_off + (q0) * 128:
                            dst_off + (q0 + n_t) * 128],
                        tr[:, 0:n_t * 128])

            def xstat(kind, m):
                src = {"a": aT, "p": pT, "q": qT}[kind]
                return src[:, m * 128:(m + 1) * 128]

            def ymov(kind, j):
                src = {"a": aT, "p": pT, "q": qT}[kind]
                if kind == "q":
                    return src[:, B:2 * B]
                return src[:, j * B:(j + 1) * B]

            def emit_pair(pidx):
                pair = PAIRS[pidx]
                W = len(pair) * B
                cs = [b for b in pair if b in es_off]
                use_stt = pidx in STT_PAIRS
                if use_stt:
                    assert len(cs) == len(pair)
                # colsum slice within the pair tile
                if len(cs) == len(pair):
                    c0, c1 = 0, W
                else:
                    h = pair.index(cs[0])
                    c0, c1 = h * B, (h + 1) * B
                esum = None
                e_first = None
                for m in range(MT):
                    mm = mmp.tile([128, W], F32, tag="mm")
                    for h, bname in enumerate(pair):
                        sk, mk, j = _blk(bname)
                        ym = ymov(mk, j)
                        for c in range(2):
                            # matmul output must fit one PSUM bank (512 f32)
                            nc.tensor.matmul(
                                mm[:, h * B + c * 512:h * B + (c + 1) * 512],
                                xstat(sk, m),
                                ym[:, c * 512:(c + 1) * 512],
                                start=True, stop=True,
                            )
                    for h, bname in enumerate(pair):
                        if bname in ("aa0", "pp0"):
                            sl = slice(h * B + m * 128, h * B + (m + 1) * 128)
                            nc.vector.tensor_add(mm[:, sl], mm[:, sl], msk[:])
                    e = ep.tile([128, W], BF16, tag="e")
                    col = st[:, m * NSTREAM + pidx: m * NSTREAM + pidx + 1]
                    if (pidx, m) in SCH_TILES:
                        nc.vector.tensor_scalar(
                            e[:].bitcast(I16), mm[:], SCH_MUL, SCH_BIAS,
                            op0=mybir.AluOpType.mult, op1=mybir.AluOpType.add)
                        if not use_stt:
                            # rowsum of e via (e*0)+e with fused accumulator
                            # (plain tensor_scalar + accum_out fails the
                            # walrus verifier)
                            nc.vector.scalar_tensor_tensor(
                                dummy[:, 0:W], e[:], 0.0, e[:],
                                op0=mybir.AluOpType.mult,
                                op1=mybir.AluOpType.add, accum_out=col)
                    else:
                        nc.scalar.activation(
                            e[:], mm[:], mybir.ActivationFunctionType.Exp,
                            scale=INV_T,
                            accum_out=None if use_stt else col)
                    if use_stt:
                        # esum accumulation with fused running rowsums: the
                        # accumulator carries sum_j esum_m[p, j]; the host
                        # diffs consecutive m to recover per-row sums.
                        if m == 0:
                            esum = sump.tile([128, W], BF16, tag="es")
                            nc.vector.scalar_tensor_tensor(
                                esum[:], e[:], 0.0, e[:],
                                op0=mybir.AluOpType.mult,
                                op1=mybir.AluOpType.add, accum_out=col)
                        else:
                            nc.vector.scalar_tensor_tensor(
                                esum[:], e[:], 1.0, esum[:],
                                op0=mybir.AluOpType.mult,
                                op1=mybir.AluOpType.add, accum_out=col)
                    elif cs:
                        if m == 0:
                            e_first = e
                        elif m == 1:
                            esum = sump.tile([128, c1 - c0], BF16, tag="es")
                            nc.vector.tensor_add(esum[:], e_first[:, c0:c1],
                                                 e[:, c0:c1])
                        else:
                            nc.vector.tensor_add(esum[:], esum[:],
                                                 e[:, c0:c1])
                if cs:
                    for i, b in enumerate(cs):
                        nc.sync.dma_start(
                            out=o_es[:, es_off[b]:es_off[b] + B],
                            in_=esum[:, i * B:(i + 1) * B])

            # transposes interleaved so each pair's operands are ready just
            # ahead of its sims while ACT lags behind on exp work
            emit_T(aT, an_nat, 0, 16, 0)
            emit_pair(0)                       # aa0, aa1
            emit_T(aT, an_nat, 16, 16, 16 * 128)
            emit_pair(1)                       # aa2, aa3
            emit_T(pT, pn_nat, 0, 16, 0)
            emit_pair(2)                       # pp0, pp1
            emit_T(pT, pn_nat, 16, 16, 16 * 128)
            emit_pair(3)                       # pp2, pp3
            emit_T(qT, qn_nat, 0, 16, 0)
            emit_T(pT, pn_nat, 32, 16, 32 * 128)
            emit_pair(4)                       # q4, ap0
            emit_T(pT, pn_nat, 48, 16, 48 * 128)
            emit_pair(5)                       # ap1, ap2
            emit_pair(6)                       # ap3, ap4
            emit_pair(7)                       # ap5, ap6
            emit_pair(8)                       # ap7

            nc.sync.dma_start(out=o_st[:], in_=st[:])
    return nc


def _split_waits(nc):
    """Walrus codegen allows ~1 sync wait per instruction; hoist extra
    waits onto same-engine NoOps inserted just before the instruction."""
    for fn in nc.m.functions:
        for blk in fn.blocks:
            new = []
            for inst in blk.instructions:
                si = getattr(inst, "sync_info", None)
                keep = 1
                if si is not None and si.on_wait and len(si.on_wait) > keep:
                    waits = list(si.on_wait)
                    for i, w in enumerate(waits[:-keep]):
                        nop = mybir.InstNoOp(name=f"{inst.name}-sw{i}")
                        nop.engine = inst.engine
                        nop.sync_info = mybir.SyncInfo(on_wait=[w], on_update=[])
                        new.append(nop)
                    inst.sync_info = mybir.SyncInfo(
                        on_wait=list(waits[-keep:]),
                        on_update=list(si.on_update))
                new.append(inst)
            blk.instructions = new


_NC_CACHE = None


def _get_program():
    global _NC_CACHE
    if _NC_CACHE is None:
        _NC_CACHE = build_program()
        mybir.codegen_inst_isa_subclasses(_NC_CACHE)
        _split_waits(_NC_CACHE)
    return _NC_CACHE


def _normalize(x):
    x = np.asarray(x, dtype=np.float64)
    return x / np.linalg.norm(x, axis=1, keepdims=True)


def prepare_inputs(a, p):
    """Host prep: normalize, cast bf16, rotate per core."""
    an = _normalize(a).astype(ml_dtypes.bfloat16)
    pn = _normalize(p).astype(ml_dtypes.bfloat16)
    in_maps = []
    for k in range(NCORES):
        ar = np.roll(an, -k * B, axis=0)
        pr = np.roll(pn, -k * B, axis=0)
        q = ar if k < 4 else pr
        qn = np.concatenate([q[0:B], q[4 * B:5 * B]], axis=0)
        in_maps.append({
            "an": np.ascontiguousarray(ar[0:4 * B]),
            "pn": np.ascontiguousarray(pr),
            "qn": np.ascontiguousarray(qn),
        })
    return in_maps


def combine(core_outs, a, p):
    """Assemble the loss from per-core rowsum/esum outputs + host math."""
    anf = _normalize(a)
    pnf = _normalize(p)
    pos_logit = INV_T * np.einsum("ij,ij->i", anf, pnf)

    partition = np.zeros(N, np.float64)
    for k, o in enumerate(core_outs):
        # rowsums: st[p, m*NSTREAM+s] -> local row m*128+p; sum streams.
        # STT streams are cumulative over m -> diff to per-m rowsums.
        stv = np.asarray(o["st"], np.float64).reshape(128, MT, NSTREAM).copy()
        for s in STT_PAIRS:
            stv[:, :, s] = np.diff(stv[:, :, s], axis=1,
                                   prepend=np.zeros((128, 1)))
        rows = stv.sum(-1).T.reshape(B)       # [local row]
        sl = np.arange(k * B, (k + 1) * B) % N
        partition[sl] += rows
        # colsums: es[:, blk] -> sum over 128 partitions -> per-column sums
        es = np.asarray(o["es"], np.float64).reshape(128,
                                                     len(COLSUM_BLOCKS), B)
        cols = es.sum(0)                      # [blk, B]
        for i, b in enumerate(COLSUM_BLOCKS):
            j = _blk(b)[2]
            g = np.arange((k + j) * B, (k + j + 1) * B) % N
            partition[g] += cols[i]

    loss = -(pos_logit - np.log(partition)).mean()
    return np.float32(loss)


def run(anchor_embeddings, positive_embeddings, trace=False, **trace_kwargs):
    a = np.ascontiguousarray(anchor_embeddings, dtype=np.float32)
    p = np.ascontiguousarray(positive_embeddings, dtype=np.float32)
    in_maps = prepare_inputs(a, p)
    nc = _get_program()
    res = run_bass_kernel_spmd(nc, in_maps, list(range(NCORES)), trace=trace,
                               **trace_kwargs)
    return combine(res.results, a, p), res


def kernel(anchor_embeddings, positive_embeddings):
    loss, _ = run(anchor_embeddings, positive_embeddings)
    return loss


# revision 15
# speedup vs baseline: 1.1160x; 1.1160x over previous
"""GTE contrastive loss kernel for 8 Trainium2 NeuronCores.

Math (reference): loss = -mean_i( cos(a_i,p_i)/T - log(partition_i) ),
partition_i = rowsum_i(E_ap) + rowsum_i(E_aa) + colsum_i(E_ap)
            + colsum_i(E_pp) - 2*exp(1/T),   E_xy = exp(cos/T).

Device does only the O(N^2) work: bf16 sims on PE, exp on ACT (plus a
tunable slice on DVE via the Schraudolph int16 bit-trick), per-row sums
via the ACT accumulator, and bf16 esum compression (sum over the 8 row
tiles of each block) so the host can finish column sums.  Everything
O(N*D) — normalization, the pos_logit diagonal, colsum reduction over
partitions, log/mean — runs on the host.

Sharding: core k owns row block k (1024 rows); inputs are rotated by
-1024k rows so one SPMD program serves all cores.  Per core, 17 block
matrices of [1024, 1024]: 8 ap, aa 0-3, pp 0-3, and "block 4" of aa
(cores 0-3) or pp (cores 4-7), fed via a third input q so the program
stays identical across cores.  Symmetry routing: rowsums of aa/pp
blocks 1-3 are emitted per core, their colsums serve the other cores'
missing blocks 5-7; block 0 is the masked diagonal block (rowsum only);
block 4 is computed once globally per unordered pair, colsum serving
the opposite core's rowsum.
"""

import os
import sys

import numpy as np

for _p in ("/opt/trn_rl_repo", os.path.expanduser("/root/.axon_site/_ro/trn_rl_repo")):
    if os.path.isdir(_p) and _p not in sys.path:
        sys.path.insert(0, _p)

import ml_dtypes  # noqa: E402

from concourse import bass, masks, tile  # noqa: E402
from concourse.bass_utils import run_bass_kernel_spmd  # noqa: E402

mybir = bass.mybir
F32 = mybir.dt.float32
BF16 = mybir.dt.bfloat16
I16 = mybir.dt.int16

N, D, NCORES = 8192, 64, 8
B = N // NCORES            # 1024 rows per core
MT = B // 128              # 8 row tiles of 128
INV_T = 20.0

# Schraudolph exp on DVE: bf16 bits = round(logit * 128/ln2 + BIAS).
# BIAS offset -7.5 zeroes the mean relative error of exp-sums (calibrated
# against round-to-nearest int16 conversion semantics measured on device).
SCH_MUL = float(INV_T * 128.0 / np.log(2.0))
SCH_BIAS = 127.0 * 128.0 - 7.5

# Per-core job list: 8 paired column blocks + 1 single.  Each entry is a
# tuple of block names; "aaJ"/"ppJ" = block J of aa/pp, "apJ" = block J
# of ap, "q4" = block 4 of aa-or-pp via the q input.
PAIRS = [
    ("aa0", "aa1"),
    ("aa2", "aa3"),
    ("pp0", "pp1"),
    ("pp2", "pp3"),
    ("q4", "ap0"),
    ("ap1", "ap2"),
    ("ap3", "ap4"),
    ("ap5", "ap6"),
    ("ap7",),
]
NSTREAM = len(PAIRS)       # 9 rowsum streams

# Blocks whose colsums the host needs, in es-output order.
COLSUM_BLOCKS = ["aa1", "aa2", "aa3", "pp1", "pp2", "pp3", "q4",
                 "ap0", "ap1", "ap2", "ap3", "ap4", "ap5", "ap6", "ap7"]
ES_W = len(COLSUM_BLOCKS) * B

# (pair_idx, m) tiles whose exp runs on DVE (Schraudolph) instead of ACT.
# Tunable: offload enough to balance ACT vs DVE busy time.
SCH_TILES = {(7, m) for m in range(MT)} | {(6, m) for m in range(4, MT)}
# aa0/pp0 carry the masked diagonal; keep those pairs on ACT (the -100
# mask would overflow the int16 Schraudolph path).
assert all(p not in (0, 2) for (p, m) in SCH_TILES)

# Pairs whose rowsums come from the esum accumulator (scalar_tensor_tensor
# accum_out = running rowsum of the esum; host diffs consecutive m) instead
# of the ACT accumulator.  Requires the esum to span the full pair width,
# i.e. every block in the pair emits a colsum.  STT runs at DVE 1x, so this
# only pays off on pairs that also carry Schraudolph tiles (it is the only
# way those tiles get rowsums).
STT_PAIRS = {6, 7}
assert {p for (p, m) in SCH_TILES} <= STT_PAIRS


def _blk(name):
    """(stat_kind, mov_kind, j): operand sources + column block index."""
    if name == "q4":
        return ("q", "q", 4)
    j = int(name[-1])
    if name.startswith("aa"):
        return ("a", "a", j)
    if name.startswith("pp"):
        return ("p", "p", j)
    return ("a", "p", j)  # apJ


def build_program():
    nc = bass.Bass()
    # Host-normalized, host-transposed bf16 operands [D, tokens], rotated
    # by -1024k rows per core.
    aT_in = nc.declare_dram_parameter("aT", [D, 4 * B], BF16, isOutput=False)
    pT_in = nc.declare_dram_parameter("pT", [D, N], BF16, isOutput=False)
    qT_in = nc.declare_dram_parameter("qT", [D, 2 * B], BF16, isOutput=False)
    o_st = nc.declare_dram_parameter("st", [128, MT * NSTREAM], F32,
                                     isOutput=True)
    o_es = nc.declare_dram_parameter("es", [128, ES_W], BF16, isOutput=True)

    es_off = {b: i * B for i, b in enumerate(COLSUM_BLOCKS)}

    with tile.TileContext(nc) as tc:
        import contextlib

        with contextlib.ExitStack() as ctx:
            res = ctx.enter_context(tc.tile_pool(name="results", bufs=1))
            st = res.tile([128, MT * NSTREAM], F32)

            ident0 = res.tile([128, 128], F32)
            masks.make_identity(nc, ident0[:])
            # -100 on the diagonal: exp(20*(s-100)) == 0 in bf16, removing
            # the aa/pp self-terms on device (no e^20 cancellation on host,
            # which reduced matmul precision cannot support)
            msk = res.tile([128, 128], F32)
            nc.vector.tensor_scalar_mul(msk[:], ident0[:], -100.0)

            xp = ctx.enter_context(tc.tile_pool(name="xT", bufs=1))
            aT = xp.tile([64, 4 * B], BF16)
            pT = xp.tile([64, N], BF16)
            qT = xp.tile([64, 2 * B], BF16)

            # input DMAs, in order of first use
            nc.sync.dma_start(out=aT[:], in_=aT_in[:])
            nc.sync.dma_start(out=pT[:, 0:2 * B], in_=pT_in[:, 0:2 * B])
            nc.sync.dma_start(out=qT[:], in_=qT_in[:])
            nc.sync.dma_start(out=pT[:, 2 * B:], in_=pT_in[:, 2 * B:])

            mmp = ctx.enter_context(tc.tile_pool(name="mm", bufs=2,
                                                 space="PSUM"))
            ep = ctx.enter_context(tc.tile_pool(name="etile", bufs=3))
            sump = ctx.enter_context(tc.tile_pool(name="esum", bufs=2))
            dummy = res.tile([128, 2 * B], BF16)

            def xstat(kind, m):
                src = {"a": aT, "p": pT, "q": qT}[kind]
                return src[:, m * 128:(m + 1) * 128]

            def ymov(kind, j):
                src = {"a": aT, "p": pT, "q": qT}[kind]
                if kind == "q":
                    return src[:, B:2 * B]
                return src[:, j * B:(j + 1) * B]

            def emit_pair(pidx):
                pair = PAIRS[pidx]
                W = len(pair) * B
                cs = [b for b in pair if b in es_off]
                use_stt = pidx in STT_PAIRS
                if use_stt:
                    assert len(cs) == len(pair)
                # colsum slice within the pair tile
                if len(cs) == len(pair):
                    c0, c1 = 0, W
                else:
                    h = pair.index(cs[0])
                    c0, c1 = h * B, (h + 1) * B
                esum = None
                e_first = None
                for m in range(MT):
                    mm = mmp.tile([128, W], F32, tag="mm")
                    for h, bname in enumerate(pair):
                        sk, mk, j = _blk(bname)
                        ym = ymov(mk, j)
                        for c in range(2):
                            # matmul output must fit one PSUM bank (512 f32)
                            nc.tensor.matmul(
                                mm[:, h * B + c * 512:h * B + (c + 1) * 512],
                                xstat(sk, m),
                                ym[:, c * 512:(c + 1) * 512],
                                start=True, stop=True,
                            )
                    for h, bname in enumerate(pair):
                        if bname in ("aa0", "pp0"):
                            sl = slice(h * B + m * 128, h * B + (m + 1) * 128)
                            nc.vector.tensor_add(mm[:, sl], mm[:, sl], msk[:])
                    e = ep.tile([128, W], BF16, tag="e")
                    col = st[:, m * NSTREAM + pidx: m * NSTREAM + pidx + 1]
                    if (pidx, m) in SCH_TILES:
                        nc.vector.tensor_scalar(
                            e[:].bitcast(I16), mm[:], SCH_MUL, SCH_BIAS,
                            op0=mybir.AluOpType.mult, op1=mybir.AluOpType.add)
                        if not use_stt:
                            # rowsum of e via (e*0)+e with fused accumulator
                            # (plain tensor_scalar + accum_out fails the
                            # walrus verifier)
                            nc.vector.scalar_tensor_tensor(
                                dummy[:, 0:W], e[:], 0.0, e[:],
                                op0=mybir.AluOpType.mult,
                                op1=mybir.AluOpType.add, accum_out=col)
                    else:
                        nc.scalar.activation(
                            e[:], mm[:], mybir.ActivationFunctionType.Exp,
                            scale=INV_T,
                            accum_out=None if use_stt else col)
                    if use_stt:
                        # esum accumulation with fused running rowsums: the
                        # accumulator carries sum_j esum_m[p, j]; the host
                        # diffs consecutive m to recover per-row sums.
                        if m == 0:
                            esum = sump.tile([128, W], BF16, tag="es")
                            nc.vector.scalar_tensor_tensor(
                                esum[:], e[:], 0.0, e[:],
                                op0=mybir.AluOpType.mult,
                                op1=mybir.AluOpType.add, accum_out=col)
                        else:
                            nc.vector.scalar_tensor_tensor(
                                esum[:], e[:], 1.0, esum[:],
                                op0=mybir.AluOpType.mult,
                                op1=mybir.AluOpType.add, accum_out=col)
                    elif cs:
                        if m == 0:
                            e_first = e
                        elif m == 1:
                            esum = sump.tile([128, c1 - c0], BF16, tag="es")
                            nc.vector.tensor_add(esum[:], e_first[:, c0:c1],
                                                 e[:, c0:c1])
                        else:
                            nc.vector.tensor_add(esum[:], esum[:],
                                                 e[:, c0:c1])
                if cs:
                    for i, b in enumerate(cs):
                        nc.sync.dma_start(
                            out=o_es[:, es_off[b]:es_off[b] + B],
                            in_=esum[:, i * B:(i + 1) * B])

            for pidx in range(len(PAIRS)):
                emit_pair(pidx)

            nc.sync.dma_start(out=o_st[:], in_=st[:])
    return nc


def _split_waits(nc):
    """Walrus codegen allows ~1 sync wait per instruction; hoist extra
    waits onto same-engine NoOps inserted just before the instruction."""
    for fn in nc.m.functions:
        for blk in fn.blocks:
            new = []
            for inst in blk.instructions:
                si = getattr(inst, "sync_info", None)
                keep = 1
                if si is not None and si.on_wait and len(si.on_wait) > keep:
                    waits = list(si.on_wait)
                    for i, w in enumerate(waits[:-keep]):
                        nop = mybir.InstNoOp(name=f"{inst.name}-sw{i}")
                        nop.engine = inst.engine
                        nop.sync_info = mybir.SyncInfo(on_wait=[w], on_update=[])
                        new.append(nop)
                    inst.sync_info = mybir.SyncInfo(
                        on_wait=list(waits[-keep:]),
                        on_update=list(si.on_update))
                new.append(inst)
            blk.instructions = new


_NC_CACHE = None


def _get_program():
    global _NC_CACHE
    if _NC_CACHE is None:
        _NC_CACHE = build_program()
        mybir.codegen_inst_isa_subclasses(_NC_CACHE)
        _split_waits(_NC_CACHE)
    return _NC_CACHE


def _normalize(x):
    x = np.asarray(x, dtype=np.float64)
    return x / np.linalg.norm(x, axis=1, keepdims=True)


def prepare_inputs(a, p):
    """Host prep: normalize, cast bf16, rotate and transpose per core."""
    an = _normalize(a).astype(ml_dtypes.bfloat16)
    pn = _normalize(p).astype(ml_dtypes.bfloat16)
    in_maps = []
    for k in range(NCORES):
        ar = np.roll(an, -k * B, axis=0)
        pr = np.roll(pn, -k * B, axis=0)
        q = ar if k < 4 else pr
        qn = np.concatenate([q[0:B], q[4 * B:5 * B]], axis=0)
        in_maps.append({
            "aT": np.ascontiguousarray(ar[0:4 * B].T),
            "pT": np.ascontiguousarray(pr.T),
            "qT": np.ascontiguousarray(qn.T),
        })
    return in_maps


def combine(core_outs, a, p):
    """Assemble the loss from per-core rowsum/esum outputs + host math."""
    anf = _normalize(a)
    pnf = _normalize(p)
    pos_logit = INV_T * np.einsum("ij,ij->i", anf, pnf)

    partition = np.zeros(N, np.float64)
    for k, o in enumerate(core_outs):
        # rowsums: st[p, m*NSTREAM+s] -> local row m*128+p; sum streams.
        # STT streams are cumulative over m -> diff to per-m rowsums.
        stv = np.asarray(o["st"], np.float64).reshape(128, MT, NSTREAM).copy()
        for s in STT_PAIRS:
            stv[:, :, s] = np.diff(stv[:, :, s], axis=1,
                                   prepend=np.zeros((128, 1)))
        rows = stv.sum(-1).T.reshape(B)       # [local row]
        sl = np.arange(k * B, (k + 1) * B) % N
        partition[sl] += rows
        # colsums: es[:, blk] -> sum over 128 partitions -> per-column sums
        es = np.asarray(o["es"], np.float64).reshape(128,
                                                     len(COLSUM_BLOCKS), B)
        cols = es.sum(0)                      # [blk, B]
        for i, b in enumerate(COLSUM_BLOCKS):
            j = _blk(b)[2]
            g = np.arange((k + j) * B, (k + j + 1) * B) % N
            partition[g] += cols[i]

    loss = -(pos_logit - np.log(partition)).mean()
    return np.float32(loss)


def run(anchor_embeddings, positive_embeddings, trace=False, **trace_kwargs):
    a = np.ascontiguousarray(anchor_embeddings, dtype=np.float32)
    p = np.ascontiguousarray(positive_embeddings, dtype=np.float32)
    in_maps = prepare_inputs(a, p)
    nc = _get_program()
    res = run_bass_kernel_spmd(nc, in_maps, list(range(NCORES)), trace=trace,
                               **trace_kwargs)
    return combine(res.results, a, p), res


def kernel(anchor_embeddings, positive_embeddings):
    loss, _ = run(anchor_embeddings, positive_embeddings)
    return loss


# revision 17
# speedup vs baseline: 1.1843x; 1.0612x over previous
"""GTE contrastive loss kernel for 8 Trainium2 NeuronCores.

Math (reference): loss = -mean_i( cos(a_i,p_i)/T - log(partition_i) ),
partition_i = rowsum_i(E_ap) + rowsum_i(E_aa) + colsum_i(E_ap)
            + colsum_i(E_pp) - 2*exp(1/T),   E_xy = exp(cos/T).

Device does only the O(N^2) work: bf16 sims on PE, exp on ACT (plus a
tunable slice on DVE via the Schraudolph int16 bit-trick), per-row sums
via the ACT accumulator, and bf16 esum compression (sum over the 8 row
tiles of each block) so the host can finish column sums.  Everything
O(N*D) — normalization, the pos_logit diagonal, colsum reduction over
partitions, log/mean — runs on the host.

Sharding: core k owns row block k (1024 rows); inputs are rotated by
-1024k rows so one SPMD program serves all cores.  Per core, 17 block
matrices of [1024, 1024]: 8 ap, aa 0-3, pp 0-3, and "block 4" of aa
(cores 0-3) or pp (cores 4-7), fed via a third input q so the program
stays identical across cores.  Symmetry routing: rowsums of aa/pp
blocks 1-3 are emitted per core, their colsums serve the other cores'
missing blocks 5-7; block 0 is the masked diagonal block (rowsum only);
block 4 is computed once globally per unordered pair, colsum serving
the opposite core's rowsum.
"""

import os
import sys

import numpy as np

for _p in ("/opt/trn_rl_repo", os.path.expanduser("/root/.axon_site/_ro/trn_rl_repo")):
    if os.path.isdir(_p) and _p not in sys.path:
        sys.path.insert(0, _p)

import ml_dtypes  # noqa: E402

from concourse import bass, masks, tile  # noqa: E402
from concourse.bass_utils import run_bass_kernel_spmd  # noqa: E402

mybir = bass.mybir
F32 = mybir.dt.float32
BF16 = mybir.dt.bfloat16
I16 = mybir.dt.int16

N, D, NCORES = 8192, 64, 8
B = N // NCORES            # 1024 rows per core
MT = B // 128              # 8 row tiles of 128
INV_T = 20.0

# Schraudolph exp on DVE: bf16 bits = round(logit * 128/ln2 + BIAS).
# BIAS offset -7.5 zeroes the mean relative error of exp-sums (calibrated
# against round-to-nearest int16 conversion semantics measured on device).
SCH_MUL = float(INV_T * 128.0 / np.log(2.0))
SCH_BIAS = 127.0 * 128.0 - 7.5

# Per-core job list: 8 paired column blocks + 1 single.  Each entry is a
# tuple of block names; "aaJ"/"ppJ" = block J of aa/pp, "apJ" = block J
# of ap, "q4" = block 4 of aa-or-pp via the q input.
PAIRS = [
    ("aa1", "aa2"),
    ("aa3", "aa0"),
    ("pp1", "pp2"),
    ("pp3", "pp0"),
    ("q4", "ap0"),
    ("ap1", "ap2"),
    ("ap3", "ap4"),
    ("ap5", "ap6"),
    ("ap7",),
]
NSTREAM = len(PAIRS)       # 9 rowsum streams

# Blocks whose colsums the host needs, in es-output order.
COLSUM_BLOCKS = ["aa1", "aa2", "aa3", "pp1", "pp2", "pp3", "q4",
                 "ap0", "ap1", "ap2", "ap3", "ap4", "ap5", "ap6", "ap7"]
ES_W = len(COLSUM_BLOCKS) * B

# (pair_idx, m) tiles whose exp runs on DVE (Schraudolph) instead of ACT.
# Tunable: offload enough to balance ACT vs DVE busy time.
SCH_TILES = {(7, m) for m in range(MT)} | {(6, m) for m in range(4, MT)}
# aa0/pp0 carry the masked diagonal; keep those pairs on ACT (the -100
# mask would overflow the int16 Schraudolph path).
assert all(p not in (0, 2) for (p, m) in SCH_TILES)

# Pairs whose rowsums come from the esum accumulator (scalar_tensor_tensor
# accum_out = running rowsum of the esum; host diffs consecutive m) instead
# of the ACT accumulator.  Requires the esum to span the full pair width,
# i.e. every block in the pair emits a colsum.  STT runs at DVE 1x, so this
# only pays off on pairs that also carry Schraudolph tiles (it is the only
# way those tiles get rowsums).
STT_PAIRS = {6, 7}
assert {p for (p, m) in SCH_TILES} <= STT_PAIRS


def _blk(name):
    """(stat_kind, mov_kind, j): operand sources + column block index."""
    if name == "q4":
        return ("q", "q", 4)
    j = int(name[-1])
    if name.startswith("aa"):
        return ("a", "a", j)
    if name.startswith("pp"):
        return ("p", "p", j)
    return ("a", "p", j)  # apJ


def build_program():
    nc = bass.Bass()
    # Host-normalized, host-transposed bf16 operands [D, tokens], rotated
    # by -1024k rows per core.
    aT_in = nc.declare_dram_parameter("aT", [D, 4 * B], BF16, isOutput=False)
    pT_in = nc.declare_dram_parameter("pT", [D, N], BF16, isOutput=False)
    qT_in = nc.declare_dram_parameter("qT", [D, 2 * B], BF16, isOutput=False)
    o_st = nc.declare_dram_parameter("st", [128, MT * NSTREAM], F32,
                                     isOutput=True)
    o_es = nc.declare_dram_parameter("es", [128, ES_W], BF16, isOutput=True)

    es_off = {b: i * B for i, b in enumerate(COLSUM_BLOCKS)}

    with tile.TileContext(nc) as tc:
        import contextlib

        with contextlib.ExitStack() as ctx:
            res = ctx.enter_context(tc.tile_pool(name="results", bufs=1))
            st = res.tile([128, MT * NSTREAM], F32)

            ident0 = res.tile([128, 128], F32)
            masks.make_identity(nc, ident0[:])
            # -100 on the diagonal: exp(20*(s-100)) == 0 in bf16, removing
            # the aa/pp self-terms on device (no e^20 cancellation on host,
            # which reduced matmul precision cannot support)
            msk = res.tile([128, 128], F32)
            nc.vector.tensor_scalar_mul(msk[:], ident0[:], -100.0)

            xp = ctx.enter_context(tc.tile_pool(name="xT", bufs=1))
            aT = xp.tile([64, 4 * B], BF16)
            pT = xp.tile([64, N], BF16)
            qT = xp.tile([64, 2 * B], BF16)

            # input DMAs, in order of first use
            nc.sync.dma_start(out=aT[:], in_=aT_in[:])
            nc.sync.dma_start(out=pT[:, 0:2 * B], in_=pT_in[:, 0:2 * B])
            nc.sync.dma_start(out=qT[:], in_=qT_in[:])
            nc.sync.dma_start(out=pT[:, 2 * B:], in_=pT_in[:, 2 * B:])

            mmp = ctx.enter_context(tc.tile_pool(name="mm", bufs=2,
                                                 space="PSUM"))
            ep = ctx.enter_context(tc.tile_pool(name="etile", bufs=3))
            sump = ctx.enter_context(tc.tile_pool(name="esum", bufs=2))
            dummy = res.tile([128, 2 * B], BF16)

            def xstat(kind, m):
                src = {"a": aT, "p": pT, "q": qT}[kind]
                return src[:, m * 128:(m + 1) * 128]

            def ymov(kind, j):
                src = {"a": aT, "p": pT, "q": qT}[kind]
                if kind == "q":
                    return src[:, B:2 * B]
                return src[:, j * B:(j + 1) * B]

            # per-pair emission state
            class PState:
                def __init__(self, pidx):
                    self.pair = PAIRS[pidx]
                    self.W = len(self.pair) * B
                    self.cs = [b for b in self.pair if b in es_off]
                    self.use_stt = pidx in STT_PAIRS
                    if self.use_stt:
                        assert len(self.cs) == len(self.pair)
                    if len(self.cs) == len(self.pair):
                        self.c0, self.c1 = 0, self.W
                    else:
                        h = self.pair.index(self.cs[0])
                        self.c0, self.c1 = h * B, (h + 1) * B
                    self.esum = None
                    self.e_first = None

            pstates = {p: PState(p) for p in range(len(PAIRS))}
            # dedicated esum slots for the (long-lived) STT pairs
            sttsump = ctx.enter_context(
                tc.tile_pool(name="sttsum", bufs=len(STT_PAIRS)))

            def emit_tile(pidx, m):
                ps = pstates[pidx]
                pair, W, cs = ps.pair, ps.W, ps.cs
                mm = mmp.tile([128, W], F32, tag="mm")
                for h, bname in enumerate(pair):
                    sk, mk, j = _blk(bname)
                    ym = ymov(mk, j)
                    for c in range(2):
                        # matmul output must fit one PSUM bank (512 f32)
                        nc.tensor.matmul(
                            mm[:, h * B + c * 512:h * B + (c + 1) * 512],
                            xstat(sk, m),
                            ym[:, c * 512:(c + 1) * 512],
                            start=True, stop=True,
                        )
                for h, bname in enumerate(pair):
                    if bname in ("aa0", "pp0"):
                        sl = slice(h * B + m * 128, h * B + (m + 1) * 128)
                        nc.vector.tensor_add(mm[:, sl], mm[:, sl], msk[:])
                e = ep.tile([128, W], BF16, tag="e")
                col = st[:, m * NSTREAM + pidx: m * NSTREAM + pidx + 1]
                if (pidx, m) in SCH_TILES:
                    nc.vector.tensor_scalar(
                        e[:].bitcast(I16), mm[:], SCH_MUL, SCH_BIAS,
                        op0=mybir.AluOpType.mult, op1=mybir.AluOpType.add)
                    if not ps.use_stt:
                        # rowsum of e via (e*0)+e with fused accumulator
                        # (plain tensor_scalar + accum_out fails the walrus
                        # verifier)
                        nc.vector.scalar_tensor_tensor(
                            dummy[:, 0:W], e[:], 0.0, e[:],
                            op0=mybir.AluOpType.mult,
                            op1=mybir.AluOpType.add, accum_out=col)
                else:
                    nc.scalar.activation(
                        e[:], mm[:], mybir.ActivationFunctionType.Exp,
                        scale=INV_T,
                        accum_out=None if ps.use_stt else col)
                if ps.use_stt:
                    # esum accumulation with fused running rowsums: the
                    # accumulator carries sum_j esum_m[p, j]; the host
                    # diffs consecutive m to recover per-row sums.
                    if m == 0:
                        ps.esum = sttsump.tile([128, W], BF16, tag="stt")
                        nc.vector.scalar_tensor_tensor(
                            ps.esum[:], e[:], 0.0, e[:],
                            op0=mybir.AluOpType.mult,
                            op1=mybir.AluOpType.add, accum_out=col)
                    else:
                        nc.vector.scalar_tensor_tensor(
                            ps.esum[:], e[:], 1.0, ps.esum[:],
                            op0=mybir.AluOpType.mult,
                            op1=mybir.AluOpType.add, accum_out=col)
                elif cs:
                    if m == 0:
                        ps.e_first = e
                    elif m == 1:
                        ps.esum = sump.tile([128, ps.c1 - ps.c0], BF16,
                                            tag="es")
                        nc.vector.tensor_add(ps.esum[:],
                                             ps.e_first[:, ps.c0:ps.c1],
                                             e[:, ps.c0:ps.c1])
                    else:
                        nc.vector.tensor_add(ps.esum[:], ps.esum[:],
                                             e[:, ps.c0:ps.c1])
                if m == MT - 1 and cs:
                    for i, b in enumerate(cs):
                        nc.sync.dma_start(
                            out=o_es[:, es_off[b]:es_off[b] + B],
                            in_=ps.esum[:, i * B:(i + 1) * B])

            # Interleave the DVE-heavy STT/Schraudolph pairs' tiles through
            # the ACT pairs so the DVE exp chain overlaps ACT exp work
            # instead of running as a serial tail.
            act_units = [(p, m) for p in sorted(set(range(len(PAIRS)))
                                                - STT_PAIRS)
                         for m in range(MT)]
            sch_units = [(p, m) for p in sorted(STT_PAIRS)
                         for m in range(MT)]
            schedule = []
            ratio = max(1, len(act_units) // max(1, len(sch_units)))
            ai = si = 0
            while ai < len(act_units) or si < len(sch_units):
                take = min(ratio, len(act_units) - ai)
                schedule.extend(act_units[ai:ai + take])
                ai += take
                if si < len(sch_units):
                    schedule.append(sch_units[si])
                    si += 1
            for pidx, m in schedule:
                emit_tile(pidx, m)

            nc.sync.dma_start(out=o_st[:], in_=st[:])
    return nc


def _split_waits(nc):
    """Walrus codegen allows ~1 sync wait per instruction; hoist extra
    waits onto same-engine NoOps inserted just before the instruction."""
    for fn in nc.m.functions:
        for blk in fn.blocks:
            new = []
            for inst in blk.instructions:
                si = getattr(inst, "sync_info", None)
                keep = 1
                if si is not None and si.on_wait and len(si.on_wait) > keep:
                    waits = list(si.on_wait)
                    for i, w in enumerate(waits[:-keep]):
                        nop = mybir.InstNoOp(name=f"{inst.name}-sw{i}")
                        nop.engine = inst.engine
                        nop.sync_info = mybir.SyncInfo(on_wait=[w], on_update=[])
                        new.append(nop)
                    inst.sync_info = mybir.SyncInfo(
                        on_wait=list(waits[-keep:]),
                        on_update=list(si.on_update))
                new.append(inst)
            blk.instructions = new


_NC_CACHE = None


def _get_program():
    global _NC_CACHE
    if _NC_CACHE is None:
        _NC_CACHE = build_program()
        mybir.codegen_inst_isa_subclasses(_NC_CACHE)
        _split_waits(_NC_CACHE)
    return _NC_CACHE


def _normalize(x):
    x = np.asarray(x, dtype=np.float64)
    return x / np.linalg.norm(x, axis=1, keepdims=True)


def prepare_inputs(a, p):
    """Host prep: normalize, cast bf16, rotate and transpose per core."""
    an = _normalize(a).astype(ml_dtypes.bfloat16)
    pn = _normalize(p).astype(ml_dtypes.bfloat16)
    in_maps = []
    for k in range(NCORES):
        ar = np.roll(an, -k * B, axis=0)
        pr = np.roll(pn, -k * B, axis=0)
        q = ar if k < 4 else pr
        qn = np.concatenate([q[0:B], q[4 * B:5 * B]], axis=0)
        in_maps.append({
            "aT": np.ascontiguousarray(ar[0:4 * B].T),
            "pT": np.ascontiguousarray(pr.T),
            "qT": np.ascontiguousarray(qn.T),
        })
    return in_maps


def combine(core_outs, a, p):
    """Assemble the loss from per-core rowsum/esum outputs + host math."""
    anf = _normalize(a)
    pnf = _normalize(p)
    pos_logit = INV_T * np.einsum("ij,ij->i", anf, pnf)

    partition = np.zeros(N, np.float64)
    for k, o in enumerate(core_outs):
        # rowsums: st[p, m*NSTREAM+s] -> local row m*128+p; sum streams.
        # STT streams are cumulative over m -> diff to per-m rowsums.
        stv = np.asarray(o["st"], np.float64).reshape(128, MT, NSTREAM).copy()
        for s in STT_PAIRS:
            stv[:, :, s] = np.diff(stv[:, :, s], axis=1,
                                   prepend=np.zeros((128, 1)))
        rows = stv.sum(-1).T.reshape(B)       # [local row]
        sl = np.arange(k * B, (k + 1) * B) % N
        partition[sl] += rows
        # colsums: es[:, blk] -> sum over 128 partitions -> per-column sums
        es = np.asarray(o["es"], np.float64).reshape(128,
                                                     len(COLSUM_BLOCKS), B)
        cols = es.sum(0)                      # [blk, B]
        for i, b in enumerate(COLSUM_BLOCKS):
            j = _blk(b)[2]
            g = np.arange((k + j) * B, (k + j + 1) * B) % N
            partition[g] += cols[i]

    loss = -(pos_logit - np.log(partition)).mean()
    return np.float32(loss)


def run(anchor_embeddings, positive_embeddings, trace=False, **trace_kwargs):
    a = np.ascontiguousarray(anchor_embeddings, dtype=np.float32)
    p = np.ascontiguousarray(positive_embeddings, dtype=np.float32)
    in_maps = prepare_inputs(a, p)
    nc = _get_program()
    res = run_bass_kernel_spmd(nc, in_maps, list(range(NCORES)), trace=trace,
                               **trace_kwargs)
    return combine(res.results, a, p), res


def kernel(anchor_embeddings, positive_embeddings):
    loss, _ = run(anchor_embeddings, positive_embeddings)
    return loss


# revision 27
# speedup vs baseline: 1.1978x; 1.0114x over previous
"""GTE contrastive loss kernel for 8 Trainium2 NeuronCores.

Math (reference): loss = -mean_i( cos(a_i,p_i)/T - log(partition_i) ),
partition_i = rowsum_i(E_ap) + rowsum_i(E_aa) + colsum_i(E_ap)
            + colsum_i(E_pp) - 2*exp(1/T),   E_xy = exp(cos/T).

Device does only the O(N^2) work: bf16 sims on PE, exp on ACT (plus a
tunable slice on DVE via the Schraudolph int16 bit-trick), per-row sums
via the ACT accumulator, and bf16 esum compression (sum over the 8 row
tiles of each block) so the host can finish column sums.  Everything
O(N*D) — normalization, the pos_logit diagonal, colsum reduction over
partitions, log/mean — runs on the host.

Sharding: core k owns row block k (1024 rows); inputs are rotated by
-1024k rows so one SPMD program serves all cores.  Per core, 17 block
matrices of [1024, 1024]: 8 ap, aa 0-3, pp 0-3, and "block 4" of aa
(cores 0-3) or pp (cores 4-7), fed via a third input q so the program
stays identical across cores.  Symmetry routing: rowsums of aa/pp
blocks 1-3 are emitted per core, their colsums serve the other cores'
missing blocks 5-7; block 0 is the masked diagonal block (rowsum only);
block 4 is computed once globally per unordered pair, colsum serving
the opposite core's rowsum.
"""

import os
import sys

import numpy as np

for _p in ("/opt/trn_rl_repo", os.path.expanduser("/root/.axon_site/_ro/trn_rl_repo")):
    if os.path.isdir(_p) and _p not in sys.path:
        sys.path.insert(0, _p)

import ml_dtypes  # noqa: E402

from concourse import bass, masks, tile  # noqa: E402
from concourse.bass_utils import run_bass_kernel_spmd  # noqa: E402

mybir = bass.mybir
F32 = mybir.dt.float32
BF16 = mybir.dt.bfloat16
I16 = mybir.dt.int16
FP8 = mybir.dt.float8e4

N, D, NCORES = 8192, 64, 8
B = N // NCORES            # 1024 rows per core
MT = B // 128              # 8 row tiles of 128
INV_T = 20.0

# Schraudolph exp on DVE: bf16 bits = round(logit * 128/ln2 + BIAS).
# BIAS offset -7.5 zeroes the mean relative error of exp-sums (calibrated
# against round-to-nearest int16 conversion semantics measured on device).
SCH_MUL = float(INV_T * 128.0 / np.log(2.0))
SCH_BIAS = 127.0 * 128.0 - 7.5

# Per-core job list: 8 paired column blocks + 1 single.  Each entry is a
# tuple of block names; "aaJ"/"ppJ" = block J of aa/pp, "apJ" = block J
# of ap, "q4" = block 4 of aa-or-pp via the q input.
PAIRS = [
    ("aa1", "aa2"),
    ("aa3", "aa0"),
    ("pp1", "pp2"),
    ("pp3", "pp0"),
    ("q4", "ap0"),
    ("ap1", "ap2"),
    ("ap3", "ap4"),
    ("ap5", "ap6"),
    ("ap7",),
]
NSTREAM = len(PAIRS)       # 9 rowsum streams

# Blocks whose colsums the host needs, in es-output order.
COLSUM_BLOCKS = ["aa1", "aa2", "aa3", "pp1", "pp2", "pp3", "q4",
                 "ap0", "ap1", "ap2", "ap3", "ap4", "ap5", "ap6", "ap7"]
# non-STT pairs emit 4 quarter-esums per block (host sums them); STT pairs
# emit one full esum in quarter slot 0
NQ = 4
ES_W = len(COLSUM_BLOCKS) * NQ * B

# (pair_idx, m) tiles whose exp runs on DVE (Schraudolph) instead of ACT.
# Tunable: offload enough to balance ACT vs DVE busy time.
SCH_TILES = {(7, m) for m in range(MT)} | {(6, m) for m in range(4, MT)}
# aa0/pp0 carry the masked diagonal; keep those pairs on ACT (the -100
# mask would overflow the int16 Schraudolph path).
assert all(p not in (0, 2) for (p, m) in SCH_TILES)

# Pairs whose rowsums come from the esum accumulator (scalar_tensor_tensor
# accum_out = running rowsum of the esum; host diffs consecutive m) instead
# of the ACT accumulator.  Requires the esum to span the full pair width,
# i.e. every block in the pair emits a colsum.  STT runs at DVE 1x, so this
# only pays off on pairs that also carry Schraudolph tiles (it is the only
# way those tiles get rowsums).
STT_PAIRS = {6, 7}
assert {p for (p, m) in SCH_TILES} <= STT_PAIRS


def _blk(name):
    """(stat_kind, mov_kind, j): operand sources + column block index."""
    if name == "q4":
        return ("q", "q", 4)
    j = int(name[-1])
    if name.startswith("aa"):
        return ("a", "a", j)
    if name.startswith("pp"):
        return ("p", "p", j)
    return ("a", "p", j)  # apJ


def build_program():
    nc = bass.Bass()
    # Host-normalized, host-transposed fp8e4m3 operands packed for the
    # DoubleRow matmul: [32, 2, tokens] with contraction k = k32 + 32*i,
    # rotated by -1024k rows per core.
    aT_in = nc.declare_dram_parameter("aT", [32, 2 * 4 * B], FP8,
                                      isOutput=False)
    pT_in = nc.declare_dram_parameter("pT", [32, 2 * N], FP8, isOutput=False)
    qT_in = nc.declare_dram_parameter("qT", [32, 2 * 2 * B], FP8,
                                      isOutput=False)
    o_st = nc.declare_dram_parameter("st", [128, MT * NSTREAM], F32,
                                     isOutput=True)
    o_es = nc.declare_dram_parameter("es", [128, ES_W], BF16, isOutput=True)

    es_off = {b: i * NQ * B for i, b in enumerate(COLSUM_BLOCKS)}

    with tile.TileContext(nc) as tc:
        import contextlib

        with contextlib.ExitStack() as ctx:
            res = ctx.enter_context(tc.tile_pool(name="results", bufs=1))
            st = res.tile([128, MT * NSTREAM], F32)

            ident0 = res.tile([128, 128], F32)
            masks.make_identity(nc, ident0[:])
            # -100 on the diagonal: exp(20*(s-100)) == 0 in bf16, removing
            # the aa/pp self-terms on device (no e^20 cancellation on host,
            # which reduced matmul precision cannot support)
            msk = res.tile([128, 128], F32)
            nc.vector.tensor_scalar_mul(msk[:], ident0[:], -100.0)

            xp = ctx.enter_context(tc.tile_pool(name="xT", bufs=1))
            aT = xp.tile([32, 2 * 4 * B], FP8)
            pT = xp.tile([32, 2 * N], FP8)
            qT = xp.tile([32, 2 * 2 * B], FP8)

            # input DMAs, in order of first use
            nc.sync.dma_start(out=aT[:], in_=aT_in[:])
            nc.sync.dma_start(out=pT[:, 0:4 * B], in_=pT_in[:, 0:4 * B])
            nc.sync.dma_start(out=qT[:], in_=qT_in[:])
            nc.sync.dma_start(out=pT[:, 4 * B:], in_=pT_in[:, 4 * B:])

            mmp = ctx.enter_context(tc.tile_pool(name="mm", bufs=2,
                                                 space="PSUM"))
            ep = ctx.enter_context(tc.tile_pool(name="etile", bufs=3))
            sump = ctx.enter_context(tc.tile_pool(name="esum", bufs=2))
            dummy = res.tile([128, 2 * B], BF16)

            def _x3(kind):
                src = {"a": aT, "p": pT, "q": qT}[kind]
                ntok = src.shape[1] // 2
                return src[:].rearrange("k (two t) -> k two t", two=2), ntok

            def xstat(kind, m):
                x3, _ = _x3(kind)
                return x3[:, :, m * 128:(m + 1) * 128]

            def ymov(kind, j):
                x3, ntok = _x3(kind)
                if kind == "q":
                    return x3[:, :, B:2 * B]
                return x3[:, :, j * B:(j + 1) * B]

            # per-pair emission state
            class PState:
                def __init__(self, pidx):
                    self.pair = PAIRS[pidx]
                    self.W = len(self.pair) * B
                    self.cs = [b for b in self.pair if b in es_off]
                    self.use_stt = pidx in STT_PAIRS
                    if self.use_stt:
                        assert len(self.cs) == len(self.pair)
                    if len(self.cs) == len(self.pair):
                        self.c0, self.c1 = 0, self.W
                    else:
                        h = self.pair.index(self.cs[0])
                        self.c0, self.c1 = h * B, (h + 1) * B
                    self.esum = None
                    self.e_first = None

            pstates = {p: PState(p) for p in range(len(PAIRS))}
            # dedicated esum slots for the (long-lived) STT pairs
            sttsump = ctx.enter_context(
                tc.tile_pool(name="sttsum", bufs=len(STT_PAIRS)))

            def emit_tile(pidx, m):
                ps = pstates[pidx]
                pair, W, cs = ps.pair, ps.W, ps.cs
                mm = mmp.tile([128, W], F32, tag="mm")
                for h, bname in enumerate(pair):
                    sk, mk, j = _blk(bname)
                    ym = ymov(mk, j)
                    for c in range(2):
                        # matmul output must fit one PSUM bank (512 f32)
                        nc.tensor.matmul(
                            mm[:, h * B + c * 512:h * B + (c + 1) * 512],
                            xstat(sk, m),
                            ym[:, :, c * 512:(c + 1) * 512],
                            start=True, stop=True,
                            perf_mode=mybir.MatmulPerfMode.DoubleRow,
                        )
                for h, bname in enumerate(pair):
                    if bname in ("aa0", "pp0"):
                        sl = slice(h * B + m * 128, h * B + (m + 1) * 128)
                        nc.vector.tensor_add(mm[:, sl], mm[:, sl], msk[:])
                e = ep.tile([128, W], BF16, tag="e")
                col = st[:, m * NSTREAM + pidx: m * NSTREAM + pidx + 1]
                if (pidx, m) in SCH_TILES:
                    nc.vector.tensor_scalar(
                        e[:].bitcast(I16), mm[:], SCH_MUL, SCH_BIAS,
                        op0=mybir.AluOpType.mult, op1=mybir.AluOpType.add)
                    if not ps.use_stt:
                        # rowsum of e via (e*0)+e with fused accumulator
                        # (plain tensor_scalar + accum_out fails the walrus
                        # verifier)
                        nc.vector.scalar_tensor_tensor(
                            dummy[:, 0:W], e[:], 0.0, e[:],
                            op0=mybir.AluOpType.mult,
                            op1=mybir.AluOpType.add, accum_out=col)
                else:
                    nc.scalar.activation(
                        e[:], mm[:], mybir.ActivationFunctionType.Exp,
                        scale=INV_T,
                        accum_out=None if ps.use_stt else col)
                if ps.use_stt:
                    # esum accumulation with fused running rowsums: the
                    # accumulator carries sum_j esum_m[p, j]; the host
                    # diffs consecutive m to recover per-row sums.
                    if m == 0:
                        ps.esum = sttsump.tile([128, W], BF16, tag="stt")
                        nc.vector.scalar_tensor_tensor(
                            ps.esum[:], e[:], 0.0, e[:],
                            op0=mybir.AluOpType.mult,
                            op1=mybir.AluOpType.add, accum_out=col)
                    else:
                        nc.vector.scalar_tensor_tensor(
                            ps.esum[:], e[:], 1.0, ps.esum[:],
                            op0=mybir.AluOpType.mult,
                            op1=mybir.AluOpType.add, accum_out=col)
                elif cs:
                    # quarter-esums: sum tile pairs (2m, 2m+1) only; the
                    # host adds the four quarters per block
                    if m % 2 == 0:
                        ps.e_first = e
                    else:
                        q = sump.tile([128, ps.c1 - ps.c0], BF16, tag="es")
                        nc.vector.tensor_add(q[:],
                                             ps.e_first[:, ps.c0:ps.c1],
                                             e[:, ps.c0:ps.c1])
                        qi = m // 2
                        for i, b in enumerate(cs):
                            nc.sync.dma_start(
                                out=o_es[:, es_off[b] + qi * B:
                                         es_off[b] + (qi + 1) * B],
                                in_=q[:, i * B:(i + 1) * B])
                if m == MT - 1 and cs and ps.use_stt:
                    for i, b in enumerate(cs):
                        nc.sync.dma_start(
                            out=o_es[:, es_off[b]:es_off[b] + B],
                            in_=ps.esum[:, i * B:(i + 1) * B])

            # Interleave the DVE-heavy STT/Schraudolph pairs' tiles through
            # the ACT pairs so the DVE exp chain overlaps ACT exp work
            # instead of running as a serial tail.
            act_units = [(p, m) for p in sorted(set(range(len(PAIRS)))
                                                - STT_PAIRS)
                         for m in range(MT)]
            sch_units = [(p, m) for p in sorted(STT_PAIRS)
                         for m in range(MT)]
            schedule = []
            ratio = max(1, len(act_units) // max(1, len(sch_units)))
            ai = si = 0
            while ai < len(act_units) or si < len(sch_units):
                take = min(ratio, len(act_units) - ai)
                schedule.extend(act_units[ai:ai + take])
                ai += take
                if si < len(sch_units):
                    schedule.append(sch_units[si])
                    si += 1
            for pidx, m in schedule:
                emit_tile(pidx, m)

            nc.sync.dma_start(out=o_st[:], in_=st[:])
    return nc


def _split_waits(nc):
    """Walrus codegen allows ~1 sync wait per instruction; hoist extra
    waits onto same-engine NoOps inserted just before the instruction."""
    for fn in nc.m.functions:
        for blk in fn.blocks:
            new = []
            for inst in blk.instructions:
                si = getattr(inst, "sync_info", None)
                keep = 1
                if si is not None and si.on_wait and len(si.on_wait) > keep:
                    waits = list(si.on_wait)
                    for i, w in enumerate(waits[:-keep]):
                        nop = mybir.InstNoOp(name=f"{inst.name}-sw{i}")
                        nop.engine = inst.engine
                        nop.sync_info = mybir.SyncInfo(on_wait=[w], on_update=[])
                        new.append(nop)
                    inst.sync_info = mybir.SyncInfo(
                        on_wait=list(waits[-keep:]),
                        on_update=list(si.on_update))
                new.append(inst)
            blk.instructions = new


_NC_CACHE = None


def _get_program():
    global _NC_CACHE
    if _NC_CACHE is None:
        _NC_CACHE = build_program()
        mybir.codegen_inst_isa_subclasses(_NC_CACHE)
        _split_waits(_NC_CACHE)
    return _NC_CACHE


def _normalize(x):
    x = np.asarray(x, dtype=np.float64)
    return x / np.linalg.norm(x, axis=1, keepdims=True)


def _pack8(x):
    """[tokens, 64] -> fp8 [32, 2*tokens] DoubleRow packing
    (arr[k, 2t..] pairs: arr[k, (i, t)] = xT[k + 32*i, t])."""
    xT = np.ascontiguousarray(x.T).astype(ml_dtypes.float8_e4m3)
    return np.ascontiguousarray(
        np.stack([xT[0:32], xT[32:64]], axis=1).reshape(32, -1))


def prepare_inputs(a, p):
    """Host prep: normalize, quantize fp8, rotate/transpose/pack per core."""
    an = _normalize(a)
    pn = _normalize(p)
    in_maps = []
    for k in range(NCORES):
        ar = np.roll(an, -k * B, axis=0)
        pr = np.roll(pn, -k * B, axis=0)
        q = ar if k < 4 else pr
        qn = np.concatenate([q[0:B], q[4 * B:5 * B]], axis=0)
        in_maps.append({
            "aT": _pack8(ar[0:4 * B]),
            "pT": _pack8(pr),
            "qT": _pack8(qn),
        })
    return in_maps


def combine(core_outs, a, p):
    """Assemble the loss from per-core rowsum/esum outputs + host math."""
    anf = _normalize(a)
    pnf = _normalize(p)
    pos_logit = INV_T * np.einsum("ij,ij->i", anf, pnf)

    partition = np.zeros(N, np.float64)
    for k, o in enumerate(core_outs):
        # rowsums: st[p, m*NSTREAM+s] -> local row m*128+p; sum streams.
        # STT streams are cumulative over m -> diff to per-m rowsums.
        stv = np.asarray(o["st"], np.float64).reshape(128, MT, NSTREAM).copy()
        for s in STT_PAIRS:
            stv[:, :, s] = np.diff(stv[:, :, s], axis=1,
                                   prepend=np.zeros((128, 1)))
        rows = stv.sum(-1).T.reshape(B)       # [local row]
        sl = np.arange(k * B, (k + 1) * B) % N
        partition[sl] += rows
        # colsums: es[:, blk, quarter] -> sum over partitions and quarters.
        # STT blocks carry one full esum in quarter 0 (other slots are
        # never written on device -- skip them).
        stt_blocks = {b for pi in STT_PAIRS for b in PAIRS[pi]}
        es = np.asarray(o["es"], np.float64).reshape(
            128, len(COLSUM_BLOCKS), NQ, B)
        for i, b in enumerate(COLSUM_BLOCKS):
            nq = 1 if b in stt_blocks else NQ
            cols = es[:, i, 0:nq, :].sum(axis=(0, 1))
            j = _blk(b)[2]
            g = np.arange((k + j) * B, (k + j + 1) * B) % N
            partition[g] += cols

    loss = -(pos_logit - np.log(partition)).mean()
    return np.float32(loss)


def run(anchor_embeddings, positive_embeddings, trace=False, **trace_kwargs):
    a = np.ascontiguousarray(anchor_embeddings, dtype=np.float32)
    p = np.ascontiguousarray(positive_embeddings, dtype=np.float32)
    in_maps = prepare_inputs(a, p)
    nc = _get_program()
    res = run_bass_kernel_spmd(nc, in_maps, list(range(NCORES)), trace=trace,
                               **trace_kwargs)
    return combine(res.results, a, p), res


def kernel(anchor_embeddings, positive_embeddings):
    loss, _ = run(anchor_embeddings, positive_embeddings)
    return loss
